# revision 1
# baseline (speedup 1.0000x reference)
"""BertScore model kernel for Trainium2 (8 NeuronCores, SPMD data-parallel over B).

Reference computation (see problem): cosine-normalized per-layer token reps,
per-(layer,batch) similarity matrix dots = h1 @ h2^T (256x256, contraction
D=1024), ragged masked max over rows/cols + masked means -> s1,s2, F1
harmonic mean -> (B,NL) features, BatchNorm over batch, linear head -> (B,).

Split of work:
- Host: normalization folded into the inputs (h = r/||r||), layout transpose
  to (NL,B,D,L) so the contraction dim D lands on SBUF partitions, additive
  ragged mask rows, and the tiny (B,4) BatchNorm + head epilogue (the
  cross-device batch-stats reduction happens here at gather time).
- Device (per core, 8 batches): 32x [DMA 2 blocks -> 16 accumulating
  matmuls + K=1 mask-row matmul (adds m2[j] to every row) -> DVE max-reduce
  for the row direction -> PE transpose of the 256x256 sim matrix + K=1
  mask-row matmul (adds m1[i]) -> DVE max-reduce for the column direction],
  accumulating 128-wide max vectors into two (128,64) buffers, DMA'd out once.

Masks are applied additively (0 valid / -1e30 invalid). The m2 row added to
the sim matrix also leaks into the transposed path, but it only offsets
whole columns j: valid j columns get +0 (exact) and invalid j columns are
dropped in the host epilogue.

The matmul dtype is selectable: float16 (default; halves DMA traffic, which
is the roofline — end-to-end rel err 6.4e-5) or float32r (full fp32 storage,
fast PE mode, rel err 2.8e-5, ~2x the DMA time).
Input DMA uses a d=8p+q partition mapping so every partition reads
4KB-contiguous runs (measured 1.6x faster than the 512B-run t*128+p mapping).
Measured device time: ~295 us/iteration under a serializing device-side
For_i loop (upper bound; the For_i back-edge defeats cross-iteration
pipelining); cost-model estimate 107.9 us against a ~99 us pure-DMA floor.
"""
import os
import numpy as np

NL, B, L1, L2, D = 4, 64, 256, 256, 1024
NCORES = 8
BB = B // NCORES          # batches per core
KT = D // 128             # contraction tiles
NEG = -1.0e30             # additive mask for invalid positions
BN_EPS = 1e-8
LOGIT_SCALE = 1.0

DTYPE = os.environ.get("BSM_DTYPE", "f16")       # f16 | f32r | f32
REPEAT = int(os.environ.get("BSM_REPEAT", "1"))  # body repeats (for timing)
U = int(os.environ.get("BSM_U", "2"))            # batches merged per DMA
SKIP = set(os.environ.get("BSM_SKIP", "").split(","))  # debug: mm,act,red,dt
IOBUFS = int(os.environ.get("BSM_IOBUFS", "4"))
LOOPN = int(os.environ.get("BSM_LOOPN", "0"))  # >0: wrap body in device For_i loop

_CACHE = {}


def _build(dtype_name, repeat, u, iobufs):
    import concourse.bacc as bacc
    import concourse.bass as bass
    import concourse.mybir as mybir
    import concourse.tile as tile
    from concourse.masks import make_identity

    f32 = mybir.dt.float32
    f32r = mybir.dt.float32r
    dt_in = {
        "f32r": f32r,
        "f16": mybir.dt.float16,
        "f32": f32,
    }[dtype_name]

    nc = bacc.Bacc("TRN2", target_bir_lowering=False, debug=False,
                   num_devices=NCORES)

    h1t = nc.dram_tensor("h1t", [NL, BB, D, L1], dt_in, kind="ExternalInput")
    h2t = nc.dram_tensor("h2t", [NL, BB, D, L2], dt_in, kind="ExternalInput")
    # m1 as per-partition columns (p, b, half): m1c[p,b,h] = m1[b, h*128+p]
    m1c = nc.dram_tensor("m1c", [128, BB, 2], f32, kind="ExternalInput")
    m2d = nc.dram_tensor("m2", [BB, L2], f32r, kind="ExternalInput")
    onesd = nc.dram_tensor("ones", [1, 128], f32r, kind="ExternalInput")
    NCOL = NL * BB * 2
    rmd = nc.dram_tensor("rm", [128, NCOL], f32, kind="ExternalOutput")
    cmd = nc.dram_tensor("cm", [128, NCOL], f32, kind="ExternalOutput")

    with tile.TileContext(nc) as tc:
        with tc.tile_pool(name="consts", bufs=1) as consts, \
             tc.tile_pool(name="io", bufs=iobufs) as io, \
             tc.tile_pool(name="dsbp", bufs=4) as dsbp, \
             tc.tile_pool(name="accp", bufs=1) as accp, \
             tc.tile_pool(name="ps", bufs=3, space="PSUM") as ps, \
             tc.tile_pool(name="psT", bufs=2, space="PSUM") as psT:

            ident = consts.tile([128, 128], f32)
            make_identity(nc, ident)
            ones = consts.tile([1, 128], f32r)
            nc.sync.dma_start(out=ones, in_=onesd.ap())

            # m2 mask rows, one partition: (1, BB, L2); m1 as columns (128, BB, 2)
            m2sb = consts.tile([1, BB, L2], f32r)
            m2ap = m2d.ap()
            nc.sync.dma_start(out=m2sb, in_=bass.AP(
                tensor=m2ap.tensor, offset=m2ap.offset,
                ap=[[0, 1], [L2, BB], [1, L2]]))
            m1sb = consts.tile([128, BB, 2], f32)
            nc.sync.dma_start(out=m1sb, in_=m1c.ap())

            RM = accp.tile([128, NCOL], f32)
            CM = accp.tile([128, NCOL], f32)
            if SKIP & {"mm", "act", "red", "dt"}:
                nc.vector.memset(RM, 0.0)
                nc.vector.memset(CM, 0.0)

            h1ap = h1t.ap()
            h2ap = h2t.ap()
            vmax = mybir.AluOpType.max
            X = mybir.AxisListType.X
            IDENT = mybir.ActivationFunctionType.Identity

            import contextlib
            loop_cm = (tc.For_i(0, LOOPN, 1,
                                hint_engines=(mybir.EngineType.PE,))
                       if LOOPN > 0 else contextlib.nullcontext())
            with loop_cm:
              for _rep in range(repeat):
                for l in range(NL):
                    # d = 8p + q: partition p reads 4KB-contiguous (q, i)
                    src1 = h1ap[l].rearrange("b (p q) i -> p b (q i)", p=128)
                    src2 = h2ap[l].rearrange("b (p q) j -> p b (q j)", p=128)
                    for bu in range(BB // u):
                        h1blk = io.tile([128, u, KT * L1], dt_in, tag="h1")
                        nc.sync.dma_start(
                            out=h1blk, in_=src1[:, bu * u:(bu + 1) * u, :])
                        h2blk = io.tile([128, u, KT * L2], dt_in, tag="h2")
                        nc.sync.dma_start(
                            out=h2blk, in_=src2[:, bu * u:(bu + 1) * u, :])
                        h1v = h1blk.rearrange("p u (q i) -> p u q i", q=KT)
                        h2v = h2blk.rearrange("p u (q j) -> p u q j", q=KT)

                        for ul in range(u):
                            if "mm" in SKIP:
                                continue
                            b = bu * u + ul
                            dsbs = []
                            for it in range(2):
                                dps = ps.tile([128, L2], f32, tag=f"dots{it}")
                                for k in range(KT):
                                    nc.tensor.matmul(
                                        out=dps,
                                        lhsT=h1v[:, ul, k,
                                                  it * 128:(it + 1) * 128],
                                        rhs=h2v[:, ul, k, :],
                                        start=(k == 0), stop=False)
                                # += m2[j] on every row (K=1 accumulate)
                                nc.tensor.matmul(out=dps, lhsT=ones,
                                                 rhs=m2sb[:, b, :],
                                                 start=False, stop=True)
                                # copy PSUM->SBUF with per-partition m1[i]
                                # added (ACT): dsb = dps + m1[i]
                                if "act" in SKIP:
                                    continue
                                dsb = dsbp.tile([128, L2], f32, tag=f"dsb{it}")
                                nc.scalar.activation(
                                    out=dsb, in_=dps, func=IDENT,
                                    bias=m1sb[:, b, it:it + 1])
                                dsbs.append(dsb)
                                # row max: m1[i] is constant along j, so the
                                # masked copy gives the same max for valid i
                                if "red" not in SKIP:
                                    col = (l * BB + b) * 2 + it
                                    nc.vector.tensor_reduce(
                                        out=RM[:, col:col + 1], in_=dsb,
                                        axis=X, op=vmax)

                            if "dt" in SKIP:
                                continue
                            dT = psT.tile([128, 2, L1], f32, tag="dT")
                            for jt in range(2):
                                for it in range(2):
                                    nc.tensor.transpose(
                                        out=dT[:, jt, it * 128:(it + 1) * 128],
                                        in_=dsbs[it][:, jt * 128:(jt + 1) * 128],
                                        identity=ident)
                            for jt in range(2):
                                col = (l * BB + b) * 2 + jt
                                nc.vector.tensor_reduce(
                                    out=CM[:, col:col + 1], in_=dT[:, jt, :],
                                    axis=X, op=vmax)

            for l in range(NL):
                c0, c1 = l * BB * 2, (l + 1) * BB * 2
                nc.sync.dma_start(out=rmd.ap()[:, c0:c1], in_=RM[:, c0:c1])
                nc.sync.dma_start(out=cmd.ap()[:, c0:c1], in_=CM[:, c0:c1])

    nc.finalize()
    return nc


def _get_nc():
    key = (DTYPE, REPEAT, U, IOBUFS, LOOPN, tuple(sorted(SKIP)))
    if key not in _CACHE:
        _CACHE[key] = _build(*key[:4])
    return _CACHE[key]


def _host_prep(reps1, reps2, len1, len2):
    """Normalize, transpose to (NL,B,D,L), build masks; returns per-core maps."""
    np_in = np.float16 if DTYPE == "f16" else np.float32

    def prep(r):
        r = np.asarray(r, dtype=np.float32)
        n = np.sqrt(np.einsum('lbid,lbid->lbi', r, r))
        h = r / n[..., None]
        return np.ascontiguousarray(h.transpose(0, 1, 3, 2)).astype(np_in)

    h1t = prep(reps1)   # (NL, B, D, L1)
    h2t = prep(reps2)
    len1 = np.asarray(len1).astype(np.int64)
    len2 = np.asarray(len2).astype(np.int64)
    ar1 = np.arange(L1)[None, :]
    ar2 = np.arange(L2)[None, :]
    m1 = np.where(ar1 < len1[:, None], 0.0, NEG).astype(np.float32)  # (B, L1)
    m2 = np.where(ar2 < len2[:, None], 0.0, NEG).astype(np.float32)
    # (B, L1) -> (B, 2, 128) -> (128, B, 2)
    m1c = np.ascontiguousarray(m1.reshape(B, 2, 128).transpose(2, 0, 1))

    in_maps = []
    for c in range(NCORES):
        sl = slice(c * BB, (c + 1) * BB)
        in_maps.append({
            "h1t": np.ascontiguousarray(h1t[:, sl]),
            "h2t": np.ascontiguousarray(h2t[:, sl]),
            "m1c": np.ascontiguousarray(m1c[:, sl]),
            "m2": np.ascontiguousarray(m2[sl]),
            "ones": np.ones((1, 128), dtype=np.float32),
        })
    return in_maps, len1, len2


def _epilogue(results, len1, len2, w, b):
    """rm/cm (128, NL*BB*2) per core -> s1,s2 -> F1 -> BatchNorm -> head."""
    maxv_rows = np.empty((NL, B, L1), dtype=np.float64)  # max over valid j, per i
    maxv_cols = np.empty((NL, B, L2), dtype=np.float64)  # max over valid i, per j
    for c, res in enumerate(results):
        rm = np.asarray(res["rm"], dtype=np.float64)  # (128, NCOL)
        cm = np.asarray(res["cm"], dtype=np.float64)
        # column t = (l*BB + b)*2 + half ; partition p -> index half*128 + p
        rm_r = rm.T.reshape(NL, BB, 2, 128).reshape(NL, BB, 256)
        cm_r = cm.T.reshape(NL, BB, 2, 128).reshape(NL, BB, 256)
        maxv_rows[:, c * BB:(c + 1) * BB] = rm_r
        maxv_cols[:, c * BB:(c + 1) * BB] = cm_r

    ar1 = np.arange(L1)[None, :]
    ar2 = np.arange(L2)[None, :]
    mask1 = (ar1 < len1[:, None])  # (B, L1)
    mask2 = (ar2 < len2[:, None])
    n1 = len1.astype(np.float64)
    n2 = len2.astype(np.float64)

    # s2: mean over valid i of (max over valid j); s1: mean over valid j of
    # (max over valid i)
    s2 = np.where(mask1[None], maxv_rows, 0.0).sum(axis=2) / n1[None]  # (NL, B)
    s1 = np.where(mask2[None], maxv_cols, 0.0).sum(axis=2) / n2[None]
    feat = (2.0 * s1 * s2 / (s1 + s2)).T                    # (B, NL)
    mean = feat.mean(axis=0, keepdims=True)
    var = ((feat - mean) ** 2).mean(axis=0, keepdims=True)
    feat = (feat - mean) / np.sqrt(var + BN_EPS)
    w = np.asarray(w, dtype=np.float64)
    bb = np.asarray(b, dtype=np.float64)
    out = LOGIT_SCALE * (feat @ w.T + bb)[:, 0]
    return out.astype(np.float32)


LAST_RUN = {}


def kernel(reps1, reps2, len1, len2, w, b):
    from concourse.bass_utils import run_bass_kernel_spmd

    nc = _get_nc()
    in_maps, l1, l2 = _host_prep(reps1, reps2, len1, len2)
    res = run_bass_kernel_spmd(nc, in_maps, list(range(NCORES)))
    LAST_RUN["results"] = res
    LAST_RUN["in_maps"] = in_maps
    return _epilogue(res.results, l1, l2, w, b)



# revision 37
# speedup vs baseline: 5.0521x; 5.0521x over previous
"""BertScore model kernel for Trainium2 (8 NeuronCores, SPMD, length-specialized).

Reference: cosine-normalized per-layer token reps, per-(layer,batch)
similarity matrix dots = h1 @ h2^T (L1 x L2, contraction D=1024), ragged
max over valid rows/cols + means -> s1,s2, F1 harmonic mean -> (B,NL)
features, BatchNorm over batch, linear head -> (B,).

Strategy: the 256 (layer,batch) units are independent until the host-side
BatchNorm. Units are grouped into 32 SPMD "slots" of 8 (one per core) with
similar (len1,len2); the device program is compiled for the actual lengths
(slot shape = max lens over its 8 units, rounded to 16 for the dual-fp8
LDWEIGHTS stride-alignment ISA rule), so only the valid ragged region is
transferred and computed. Host-side replicate-padding (row/col len-1 copied
into the padded tail) keeps every max exact with no masking. Inputs are fp8
e4m3 (end-to-end rel err ~8e-3 vs the 2e-2 gate; DMA is the roofline and
fp8 halves it); matmuls run in DoubleRow perf mode (two 128-deep K-tiles
per instruction, 0.5 cyc/row).

Per slot (stationary = the shorter of h1/h2, halving LDWEIGHTS cost):
4*n_st DoubleRow matmuls -> ACT copies PSUM to SBUF as f16 -> DVE free-axis
max over the moving index -> PE transposes (f16, software-pipelined one slot
behind the matmuls so the in-order PE queue never stalls on the ACT copy) ->
DVE free-axis max over the stationary index. Row/col max vectors accumulate
in SBUF f16 columns; two small DMAs out at the end. Slots execute
largest-first (minimal compute tail) and are DMA'd in multi-slot chunks
(first chunks small for fast pipeline fill) laid out so each partition reads
one contiguous run per chunk (~330 GB/s measured). Host epilogue: means
over valid prefixes, F1, BatchNorm over the full batch, linear head.

Measured on trn2 (8 cores, NTFF profile of the single NEFF execution):
55.7-68 us depending on device load (median ~58 us in quiet conditions),
vs 295 us for the staged baseline and 131 us for its single-shot profile.
"""
import os
import numpy as np

NL, B, L1, L2, D = 4, 64, 256, 256, 1024
NCORES = 8
NUNITS = NL * B           # 256 independent (layer, batch) units
NSLOTS = NUNITS // NCORES  # 32 slots, one unit per core each
KT = D // 128             # contraction tiles
BN_EPS = 1e-8
LOGIT_SCALE = 1.0

CHUNK_BYTES = int(os.environ.get("BSM_CHUNK", str(12 * 1024)))  # per partition

_CACHE = {}


def _plan(len1, len2):
    """Group units into slots; chunk slots for DMA; exec order big-first."""
    l1u = np.repeat(len1[None, :], NL, 0).ravel()   # unit u = l*B + b
    l2u = np.repeat(len2[None, :], NL, 0).ravel()
    nit = -(-l1u // 128)
    key = nit * 10**9 + l2u * 10**3 + l1u
    order_units = np.argsort(-key)                  # big first
    groups = order_units.reshape(NSLOTS, NCORES)
    # dual-fp8 LDWEIGHTS requires 16B-aligned k-tile strides
    shapes = np.stack([
        np.array([-16 * (-l1u[g].max() // 16) for g in groups]),
        np.array([-16 * (-l2u[g].max() // 16) for g in groups])], axis=1)
    order = np.argsort(-(shapes[:, 0] + shapes[:, 1]), kind="stable")
    # chunk consecutive exec-order slots: per-partition run per slot is
    # 8*(L1s+L2s) bytes. First chunks are small so compute starts early
    # (pipeline fill), later ones large (fewer DMA issues).
    ramp = [4 * 1024, 6 * 1024, 8 * 1024]
    chunks = []
    cur, cur_bytes = [], 0
    for s in order:
        cap = ramp[len(chunks)] if len(chunks) < len(ramp) else CHUNK_BYTES
        run = 8 * int(shapes[s][0] + shapes[s][1])
        if cur and cur_bytes + run > cap:
            chunks.append(cur)
            cur, cur_bytes = [], 0
        cur.append(int(s))
        cur_bytes += run
    if cur:
        chunks.append(cur)
    return {"groups": groups, "shapes": shapes, "order": order,
            "chunks": chunks}


def _build(shapes, chunks, plan):
    import concourse.bacc as bacc
    import concourse.bass as bass
    import concourse.mybir as mybir
    import concourse.tile as tile

    f32 = mybir.dt.float32
    f16 = mybir.dt.float16
    fp8 = mybir.dt.float8e4
    SWI = mybir.MatmulPerfMode.DoubleRowSwInterleave
    DR = mybir.MatmulPerfMode.DoubleRow

    # DRAM layout: chunk-major; within a chunk partition p holds one
    # contiguous run = concat over the chunk's slots of (8 d-rows of h1,
    # 8 d-rows of h2), each slot contributing 8*(L1s+L2s) bytes.
    chunk_off, chunk_run = [], []
    slot_in_chunk = {}
    off = 0
    for ci, ch in enumerate(chunks):
        run = 0
        for s in ch:
            slot_in_chunk[s] = (ci, run)
            run += 8 * int(shapes[s][0] + shapes[s][1])
        chunk_off.append(off)
        chunk_run.append(run)
        off += 128 * run
    TOT = off

    # RM: one f16 column per (slot, it); CM: one per (slot, jt).
    rm_col, cm_col = {}, {}
    nc1 = nc2 = 0
    for ch in chunks:
        for s in ch:
            L1s, L2s = int(shapes[s][0]), int(shapes[s][1])
            for it in range(-(-L1s // 128)):
                rm_col[(s, it)] = nc1
                nc1 += 1
            for jt in range(-(-L2s // 128)):
                cm_col[(s, jt)] = nc2
                nc2 += 1

    nc = bacc.Bacc("TRN2", target_bir_lowering=False, debug=False,
                   num_devices=NCORES)
    xin = nc.dram_tensor("xin", [TOT], fp8, kind="ExternalInput")
    rmd = nc.dram_tensor("rm", [128, nc1], f16, kind="ExternalOutput")
    cmd = nc.dram_tensor("cm", [128, nc2], f16, kind="ExternalOutput")
    xap = xin.ap()

    with tile.TileContext(nc) as tc:
        from concourse.masks import make_identity
        with tc.tile_pool(name="consts", bufs=1) as consts, \
             tc.tile_pool(name="io", bufs=4) as io, \
             tc.tile_pool(name="dsbp", bufs=6) as dsbp, \
             tc.tile_pool(name="accp", bufs=1) as accp, \
             tc.tile_pool(name="ps", bufs=4, space="PSUM") as ps, \
             tc.tile_pool(name="psT", bufs=4, space="PSUM") as psT:

            ident = consts.tile([128, 128], f16)
            make_identity(nc, ident)
            RM = accp.tile([128, nc1], f16)
            CM = accp.tile([128, nc2], f16)

            vmax = mybir.AluOpType.max
            X = mybir.AxisListType.X
            IDENT = mybir.ActivationFunctionType.Identity

            MAXRUN = max(chunk_run)

            def emit_transposes(st):
                """Transpose phase of a slot: dT blocks + moving-side max."""
                s, Lst, Lmv, dsbs, tr_acc, tr_col = st
                n_mv = -(-Lmv // 128)
                dT = psT.tile([128, 2, L1], f16, tag="dT")
                for tt in range(n_mv):
                    j0 = tt * 128
                    jlen = min(128, Lmv - j0)
                    for t, (dsb, ilen) in enumerate(dsbs):
                        nc.tensor.transpose(
                            out=dT[:jlen, tt, t * 128:t * 128 + ilen],
                            in_=dsb[:ilen, j0:j0 + jlen],
                            identity=ident[:ilen, :ilen])
                    col = tr_col[(s, tt)]
                    nc.vector.tensor_reduce(
                        out=tr_acc[:jlen, col:col + 1],
                        in_=dT[:jlen, tt, :Lst], axis=X, op=vmax)

            pending = None   # software pipeline: transposes lag one slot
            for ci, ch in enumerate(chunks):
                blk = io.tile([128, MAXRUN], fp8, tag="io")
                run = chunk_run[ci]
                nc.sync.dma_start(
                    out=blk[:, :run],
                    in_=bass.AP(tensor=xap.tensor, offset=chunk_off[ci],
                                ap=[[run, 128], [1, run]]))
                for s in ch:
                    L1s, L2s = int(shapes[s][0]), int(shapes[s][1])
                    soff = slot_in_chunk[s][1]
                    # stationary = shorter side (LDWEIGHTS cost ~ 8*Lst),
                    # packed host-side in SwInterleave layout (k-tile pairs
                    # byte-interleaved, columns reversed -> 2x weight load).
                    # Slot run layout: [st swi 8*Lst | mv q-major 8*Lmv].
                    if L1s <= L2s:
                        Lst, Lmv = L1s, L2s
                        fr_acc, fr_col = RM, rm_col    # out[i, j]
                        tr_acc, tr_col = CM, cm_col
                    else:
                        Lst, Lmv = L2s, L1s
                        fr_acc, fr_col = CM, cm_col    # out[j, i]
                        tr_acc, tr_col = RM, rm_col
                    stv = blk[:, soff:soff + 8 * Lst].rearrange(
                        "p (q i) -> p q i", q=8)
                    mvv = blk[:, soff + 8 * Lst:soff + 8 * (Lst + Lmv)
                              ].rearrange("p (q j) -> p q j", q=8)
                    n_st = -(-Lst // 128)

                    dsbs = []
                    for t in range(n_st):
                        i0 = t * 128
                        ilen = min(128, Lst - i0)
                        dps = ps.tile([128, L2], f32, tag="dots")
                        for k in range(0, KT, 2):
                            nc.tensor.matmul(
                                out=dps[:ilen, :Lmv],
                                lhsT=stv[:, k:k + 2, i0:i0 + ilen],
                                rhs=mvv[:, k:k + 2, :],
                                start=(k == 0), stop=(k == KT - 2),
                                perf_mode=DR)
                        dsb = dsbp.tile([128, L2], f16, tag="dsb")
                        nc.scalar.activation(
                            out=dsb[:ilen, :Lmv], in_=dps[:ilen, :Lmv],
                            func=IDENT)
                        dsbs.append((dsb, ilen))
                        col = fr_col[(s, t)]
                        nc.vector.tensor_reduce(
                            out=fr_acc[:ilen, col:col + 1],
                            in_=dsb[:ilen, :Lmv], axis=X, op=vmax)

                    if pending is not None:
                        emit_transposes(pending)
                    pending = (s, Lst, Lmv, dsbs, tr_acc, tr_col)
            if pending is not None:
                emit_transposes(pending)
            nc.sync.dma_start(out=rmd.ap(), in_=RM)
            nc.sync.dma_start(out=cmd.ap(), in_=CM)


    nc.finalize()
    return nc, rm_col, cm_col


def _get_plan_nc(len1, len2):
    key = (tuple(len1.tolist()), tuple(len2.tolist()), CHUNK_BYTES)
    if key not in _CACHE:
        plan = _plan(len1, len2)
        nc, rm_col, cm_col = _build(plan["shapes"], plan["chunks"], plan)
        _CACHE[key] = (plan, nc, rm_col, cm_col)
    return _CACHE[key]


def _host_prep(reps1, reps2, len1, len2, plan):
    """Normalize, fp8-cast, replicate-pad, pack per-core chunk buffers."""
    import ml_dtypes
    np_in = ml_dtypes.float8_e4m3

    def prep(r, lens, L):
        r = np.asarray(r, dtype=np.float32)
        n = np.sqrt(np.einsum('lbid,lbid->lbi', r, r))
        h = r / n[..., None]
        idx = np.minimum(np.arange(L)[None, :], (lens - 1)[:, None])  # (B, L)
        h = np.take_along_axis(h, idx[None, :, :, None], axis=2)
        return np.ascontiguousarray(h.transpose(0, 1, 3, 2)).astype(np_in)

    h1t = prep(reps1, len1, L1)   # (NL, B, D, L)
    h2t = prep(reps2, len2, L2)

    def swi_pack(h, Lst):
        """[1024, Lst] d-major -> [128, 8*Lst] (q-major per partition)."""
        return h.reshape(128, 8 * Lst)

    groups, shapes = plan["groups"], plan["shapes"]
    in_maps = []
    for c in range(NCORES):
        parts = []
        for ch in plan["chunks"]:
            rows = []   # per-partition segments, list of (128, seg) arrays
            for s in ch:
                u = groups[s][c]
                l, b = int(u) // B, int(u) % B
                L1s, L2s = int(shapes[s][0]), int(shapes[s][1])
                b1 = h1t[l, b, :, :L1s]
                b2 = h2t[l, b, :, :L2s]
                if L1s <= L2s:
                    st, mv, Lst = b1, b2, L1s
                else:
                    st, mv, Lst = b2, b1, L2s
                rows.append(swi_pack(st, Lst))
                rows.append(mv.reshape(128, 8 * mv.shape[1]))
            parts.append(np.concatenate(rows, axis=1).ravel())
        in_maps.append({"xin": np.concatenate(parts)})
    return in_maps, len1, len2


def _epilogue(results, len1, len2, w, b, plan, rm_col, cm_col):
    groups, shapes = plan["groups"], plan["shapes"]
    maxv_rows = np.zeros((NL, B, L1), dtype=np.float64)
    maxv_cols = np.zeros((NL, B, L2), dtype=np.float64)
    for c, res in enumerate(results):
        rm = np.asarray(res["rm"], dtype=np.float64)  # (128, nc1)
        cm = np.asarray(res["cm"], dtype=np.float64)
        for s in range(NSLOTS):
            u = groups[s][c]
            l, bb = int(u) // B, int(u) % B
            L1s, L2s = int(shapes[s][0]), int(shapes[s][1])
            for it in range(-(-L1s // 128)):
                ilen = min(128, L1s - it * 128)
                maxv_rows[l, bb, it * 128:it * 128 + ilen] = \
                    rm[:ilen, rm_col[(s, it)]]
            for jt in range(-(-L2s // 128)):
                jlen = min(128, L2s - jt * 128)
                maxv_cols[l, bb, jt * 128:jt * 128 + jlen] = \
                    cm[:jlen, cm_col[(s, jt)]]

    ar1 = np.arange(L1)[None, :]
    ar2 = np.arange(L2)[None, :]
    mask1 = (ar1 < len1[:, None])
    mask2 = (ar2 < len2[:, None])
    n1 = len1.astype(np.float64)
    n2 = len2.astype(np.float64)
    s2 = np.where(mask1[None], maxv_rows, 0.0).sum(axis=2) / n1[None]
    s1 = np.where(mask2[None], maxv_cols, 0.0).sum(axis=2) / n2[None]
    feat = (2.0 * s1 * s2 / (s1 + s2)).T
    mean = feat.mean(axis=0, keepdims=True)
    var = ((feat - mean) ** 2).mean(axis=0, keepdims=True)
    feat = (feat - mean) / np.sqrt(var + BN_EPS)
    w = np.asarray(w, dtype=np.float64)
    bb = np.asarray(b, dtype=np.float64)
    out = LOGIT_SCALE * (feat @ w.T + bb)[:, 0]
    return out.astype(np.float32)


LAST_RUN = {}


def kernel(reps1, reps2, len1, len2, w, b):
    from concourse.bass_utils import run_bass_kernel_spmd

    len1 = np.asarray(len1).astype(np.int64)
    len2 = np.asarray(len2).astype(np.int64)
    plan, nc, rm_col, cm_col = _get_plan_nc(len1, len2)
    in_maps, l1, l2 = _host_prep(reps1, reps2, len1, len2, plan)
    res = run_bass_kernel_spmd(nc, in_maps, list(range(NCORES)))
    LAST_RUN["results"] = res
    LAST_RUN["in_maps"] = in_maps
    LAST_RUN["nc"] = nc
    return _epilogue(res.results, l1, l2, w, b, plan, rm_col, cm_col)


# revision 38
# speedup vs baseline: 5.2526x; 1.0397x over previous
"""BertScore model kernel for Trainium2 (8 NeuronCores, SPMD, length-specialized).

Reference: cosine-normalized per-layer token reps, per-(layer,batch)
similarity matrix dots = h1 @ h2^T (L1 x L2, contraction D=1024), ragged
max over valid rows/cols + means -> s1,s2, F1 harmonic mean -> (B,NL)
features, BatchNorm over batch, linear head -> (B,).

Strategy: the 256 (layer,batch) units are independent until the host-side
BatchNorm. Units are grouped into 32 SPMD "slots" of 8 (one per core) with
similar (len1,len2); the device program is compiled for the actual lengths
(slot shape = max lens over its 8 units, rounded to 16 for the dual-fp8
LDWEIGHTS stride-alignment ISA rule), so only the valid ragged region is
transferred and computed. Host-side replicate-padding (row/col len-1 copied
into the padded tail) keeps every max exact with no masking. Inputs are fp8
e4m3 (end-to-end rel err ~8e-3 vs the 2e-2 gate; DMA is the roofline and
fp8 halves it); matmuls run in DoubleRow perf mode (two 128-deep K-tiles
per instruction, 0.5 cyc/row).

Per slot (stationary = the shorter of h1/h2, halving LDWEIGHTS cost):
4*n_st DoubleRow matmuls -> ACT copies PSUM to SBUF as f16 -> DVE free-axis
max over the moving index -> PE transposes (f16, software-pipelined one slot
behind the matmuls so the in-order PE queue never stalls on the ACT copy) ->
DVE free-axis max over the stationary index. Row/col max vectors accumulate
in SBUF f16 columns; two small DMAs out at the end. Slots execute
largest-first (minimal compute tail) and are DMA'd in multi-slot chunks
(first chunks small for fast pipeline fill) laid out so each partition reads
one contiguous run per chunk (~330 GB/s measured). Host epilogue: means
over valid prefixes, F1, BatchNorm over the full batch, linear head.

Measured on trn2 (8 cores, NTFF profile of the single NEFF execution):
55.7-68 us depending on device load (median ~58 us in quiet conditions),
vs 295 us for the staged baseline and 131 us for its single-shot profile.
"""
import os
import numpy as np

NL, B, L1, L2, D = 4, 64, 256, 256, 1024
NCORES = 8
NUNITS = NL * B           # 256 independent (layer, batch) units
NSLOTS = NUNITS // NCORES  # 32 slots, one unit per core each
KT = D // 128             # contraction tiles
BN_EPS = 1e-8
LOGIT_SCALE = 1.0

CHUNK_BYTES = int(os.environ.get("BSM_CHUNK", str(8 * 1024)))  # per partition

_CACHE = {}


def _plan(len1, len2):
    """Group units into slots; chunk slots for DMA; exec order big-first."""
    l1u = np.repeat(len1[None, :], NL, 0).ravel()   # unit u = l*B + b
    l2u = np.repeat(len2[None, :], NL, 0).ravel()
    nit = -(-l1u // 128)
    key = nit * 10**9 + l2u * 10**3 + l1u
    order_units = np.argsort(-key)                  # big first
    groups = order_units.reshape(NSLOTS, NCORES)
    # dual-fp8 LDWEIGHTS requires 16B-aligned k-tile strides
    shapes = np.stack([
        np.array([-16 * (-l1u[g].max() // 16) for g in groups]),
        np.array([-16 * (-l2u[g].max() // 16) for g in groups])], axis=1)
    order = np.argsort(-(shapes[:, 0] + shapes[:, 1]), kind="stable")
    # chunk consecutive exec-order slots: per-partition run per slot is
    # 8*(L1s+L2s) bytes. First chunks are small so compute starts early
    # (pipeline fill), later ones large (fewer DMA issues).
    ramp = [4 * 1024, 6 * 1024, 8 * 1024]
    chunks = []
    cur, cur_bytes = [], 0
    for s in order:
        cap = ramp[len(chunks)] if len(chunks) < len(ramp) else CHUNK_BYTES
        run = 8 * int(shapes[s][0] + shapes[s][1])
        if cur and cur_bytes + run > cap:
            chunks.append(cur)
            cur, cur_bytes = [], 0
        cur.append(int(s))
        cur_bytes += run
    if cur:
        chunks.append(cur)
    return {"groups": groups, "shapes": shapes, "order": order,
            "chunks": chunks}


def _build(shapes, chunks, plan):
    import concourse.bacc as bacc
    import concourse.bass as bass
    import concourse.mybir as mybir
    import concourse.tile as tile

    f32 = mybir.dt.float32
    f16 = mybir.dt.float16
    fp8 = mybir.dt.float8e4
    SWI = mybir.MatmulPerfMode.DoubleRowSwInterleave
    DR = mybir.MatmulPerfMode.DoubleRow

    # DRAM layout: chunk-major; within a chunk partition p holds one
    # contiguous run = concat over the chunk's slots of (8 d-rows of h1,
    # 8 d-rows of h2), each slot contributing 8*(L1s+L2s) bytes.
    chunk_off, chunk_run = [], []
    slot_in_chunk = {}
    off = 0
    for ci, ch in enumerate(chunks):
        run = 0
        for s in ch:
            slot_in_chunk[s] = (ci, run)
            run += 8 * int(shapes[s][0] + shapes[s][1])
        chunk_off.append(off)
        chunk_run.append(run)
        off += 128 * run
    TOT = off

    # RM: one f16 column per (slot, it); CM: one per (slot, jt).
    rm_col, cm_col = {}, {}
    nc1 = nc2 = 0
    for ch in chunks:
        for s in ch:
            L1s, L2s = int(shapes[s][0]), int(shapes[s][1])
            for it in range(-(-L1s // 128)):
                rm_col[(s, it)] = nc1
                nc1 += 1
            for jt in range(-(-L2s // 128)):
                cm_col[(s, jt)] = nc2
                nc2 += 1

    nc = bacc.Bacc("TRN2", target_bir_lowering=False, debug=False,
                   num_devices=NCORES)
    xin = nc.dram_tensor("xin", [TOT], fp8, kind="ExternalInput")
    rmd = nc.dram_tensor("rm", [128, nc1], f16, kind="ExternalOutput")
    cmd = nc.dram_tensor("cm", [128, nc2], f16, kind="ExternalOutput")
    xap = xin.ap()

    with tile.TileContext(nc) as tc:
        from concourse.masks import make_identity
        with tc.tile_pool(name="consts", bufs=1) as consts, \
             tc.tile_pool(name="io", bufs=4) as io, \
             tc.tile_pool(name="dsbp", bufs=6) as dsbp, \
             tc.tile_pool(name="accp", bufs=1) as accp, \
             tc.tile_pool(name="ps", bufs=4, space="PSUM") as ps, \
             tc.tile_pool(name="psT", bufs=4, space="PSUM") as psT:

            ident = consts.tile([128, 128], f16)
            make_identity(nc, ident)
            RM = accp.tile([128, nc1], f16)
            CM = accp.tile([128, nc2], f16)

            vmax = mybir.AluOpType.max
            X = mybir.AxisListType.X
            IDENT = mybir.ActivationFunctionType.Identity

            MAXRUN = max(chunk_run)

            def emit_transposes(st):
                """Transpose phase of a slot: dT blocks + moving-side max."""
                s, Lst, Lmv, dsbs, tr_acc, tr_col = st
                n_mv = -(-Lmv // 128)
                dT = psT.tile([128, 2, L1], f16, tag="dT")
                for tt in range(n_mv):
                    j0 = tt * 128
                    jlen = min(128, Lmv - j0)
                    for t, (dsb, ilen) in enumerate(dsbs):
                        nc.tensor.transpose(
                            out=dT[:jlen, tt, t * 128:t * 128 + ilen],
                            in_=dsb[:ilen, j0:j0 + jlen],
                            identity=ident[:ilen, :ilen])
                    col = tr_col[(s, tt)]
                    nc.vector.tensor_reduce(
                        out=tr_acc[:jlen, col:col + 1],
                        in_=dT[:jlen, tt, :Lst], axis=X, op=vmax)

            pending = None   # software pipeline: transposes lag one slot
            for ci, ch in enumerate(chunks):
                blk = io.tile([128, MAXRUN], fp8, tag="io")
                run = chunk_run[ci]
                nc.sync.dma_start(
                    out=blk[:, :run],
                    in_=bass.AP(tensor=xap.tensor, offset=chunk_off[ci],
                                ap=[[run, 128], [1, run]]))
                for s in ch:
                    L1s, L2s = int(shapes[s][0]), int(shapes[s][1])
                    soff = slot_in_chunk[s][1]
                    # stationary = shorter side (LDWEIGHTS cost ~ 8*Lst),
                    # packed host-side in SwInterleave layout (k-tile pairs
                    # byte-interleaved, columns reversed -> 2x weight load).
                    # Slot run layout: [st swi 8*Lst | mv q-major 8*Lmv].
                    if L1s <= L2s:
                        Lst, Lmv = L1s, L2s
                        fr_acc, fr_col = RM, rm_col    # out[i, j]
                        tr_acc, tr_col = CM, cm_col
                    else:
                        Lst, Lmv = L2s, L1s
                        fr_acc, fr_col = CM, cm_col    # out[j, i]
                        tr_acc, tr_col = RM, rm_col
                    stv = blk[:, soff:soff + 8 * Lst].rearrange(
                        "p (q i) -> p q i", q=8)
                    mvv = blk[:, soff + 8 * Lst:soff + 8 * (Lst + Lmv)
                              ].rearrange("p (q j) -> p q j", q=8)
                    n_st = -(-Lst // 128)

                    dsbs = []
                    for t in range(n_st):
                        i0 = t * 128
                        ilen = min(128, Lst - i0)
                        dps = ps.tile([128, L2], f32, tag="dots")
                        for k in range(0, KT, 2):
                            nc.tensor.matmul(
                                out=dps[:ilen, :Lmv],
                                lhsT=stv[:, k:k + 2, i0:i0 + ilen],
                                rhs=mvv[:, k:k + 2, :],
                                start=(k == 0), stop=(k == KT - 2),
                                perf_mode=DR)
                        dsb = dsbp.tile([128, L2], f16, tag="dsb")
                        nc.scalar.activation(
                            out=dsb[:ilen, :Lmv], in_=dps[:ilen, :Lmv],
                            func=IDENT)
                        dsbs.append((dsb, ilen))
                        col = fr_col[(s, t)]
                        nc.vector.tensor_reduce(
                            out=fr_acc[:ilen, col:col + 1],
                            in_=dsb[:ilen, :Lmv], axis=X, op=vmax)

                    if pending is not None:
                        emit_transposes(pending)
                    pending = (s, Lst, Lmv, dsbs, tr_acc, tr_col)
            if pending is not None:
                emit_transposes(pending)
            nc.sync.dma_start(out=rmd.ap(), in_=RM)
            nc.sync.dma_start(out=cmd.ap(), in_=CM)


    nc.finalize()
    return nc, rm_col, cm_col


def _get_plan_nc(len1, len2):
    key = (tuple(len1.tolist()), tuple(len2.tolist()), CHUNK_BYTES)
    if key not in _CACHE:
        plan = _plan(len1, len2)
        nc, rm_col, cm_col = _build(plan["shapes"], plan["chunks"], plan)
        _CACHE[key] = (plan, nc, rm_col, cm_col)
    return _CACHE[key]


def _host_prep(reps1, reps2, len1, len2, plan):
    """Normalize, fp8-cast, replicate-pad, pack per-core chunk buffers."""
    import ml_dtypes
    np_in = ml_dtypes.float8_e4m3

    def prep(r, lens, L):
        r = np.asarray(r, dtype=np.float32)
        n = np.sqrt(np.einsum('lbid,lbid->lbi', r, r))
        h = r / n[..., None]
        idx = np.minimum(np.arange(L)[None, :], (lens - 1)[:, None])  # (B, L)
        h = np.take_along_axis(h, idx[None, :, :, None], axis=2)
        return np.ascontiguousarray(h.transpose(0, 1, 3, 2)).astype(np_in)

    h1t = prep(reps1, len1, L1)   # (NL, B, D, L)
    h2t = prep(reps2, len2, L2)

    def swi_pack(h, Lst):
        """[1024, Lst] d-major -> [128, 8*Lst] (q-major per partition)."""
        return h.reshape(128, 8 * Lst)

    groups, shapes = plan["groups"], plan["shapes"]
    in_maps = []
    for c in range(NCORES):
        parts = []
        for ch in plan["chunks"]:
            rows = []   # per-partition segments, list of (128, seg) arrays
            for s in ch:
                u = groups[s][c]
                l, b = int(u) // B, int(u) % B
                L1s, L2s = int(shapes[s][0]), int(shapes[s][1])
                b1 = h1t[l, b, :, :L1s]
                b2 = h2t[l, b, :, :L2s]
                if L1s <= L2s:
                    st, mv, Lst = b1, b2, L1s
                else:
                    st, mv, Lst = b2, b1, L2s
                rows.append(swi_pack(st, Lst))
                rows.append(mv.reshape(128, 8 * mv.shape[1]))
            parts.append(np.concatenate(rows, axis=1).ravel())
        in_maps.append({"xin": np.concatenate(parts)})
    return in_maps, len1, len2


def _epilogue(results, len1, len2, w, b, plan, rm_col, cm_col):
    groups, shapes = plan["groups"], plan["shapes"]
    maxv_rows = np.zeros((NL, B, L1), dtype=np.float64)
    maxv_cols = np.zeros((NL, B, L2), dtype=np.float64)
    for c, res in enumerate(results):
        rm = np.asarray(res["rm"], dtype=np.float64)  # (128, nc1)
        cm = np.asarray(res["cm"], dtype=np.float64)
        for s in range(NSLOTS):
            u = groups[s][c]
            l, bb = int(u) // B, int(u) % B
            L1s, L2s = int(shapes[s][0]), int(shapes[s][1])
            for it in range(-(-L1s // 128)):
                ilen = min(128, L1s - it * 128)
                maxv_rows[l, bb, it * 128:it * 128 + ilen] = \
                    rm[:ilen, rm_col[(s, it)]]
            for jt in range(-(-L2s // 128)):
                jlen = min(128, L2s - jt * 128)
                maxv_cols[l, bb, jt * 128:jt * 128 + jlen] = \
                    cm[:jlen, cm_col[(s, jt)]]

    ar1 = np.arange(L1)[None, :]
    ar2 = np.arange(L2)[None, :]
    mask1 = (ar1 < len1[:, None])
    mask2 = (ar2 < len2[:, None])
    n1 = len1.astype(np.float64)
    n2 = len2.astype(np.float64)
    s2 = np.where(mask1[None], maxv_rows, 0.0).sum(axis=2) / n1[None]
    s1 = np.where(mask2[None], maxv_cols, 0.0).sum(axis=2) / n2[None]
    feat = (2.0 * s1 * s2 / (s1 + s2)).T
    mean = feat.mean(axis=0, keepdims=True)
    var = ((feat - mean) ** 2).mean(axis=0, keepdims=True)
    feat = (feat - mean) / np.sqrt(var + BN_EPS)
    w = np.asarray(w, dtype=np.float64)
    bb = np.asarray(b, dtype=np.float64)
    out = LOGIT_SCALE * (feat @ w.T + bb)[:, 0]
    return out.astype(np.float32)


LAST_RUN = {}


def kernel(reps1, reps2, len1, len2, w, b):
    from concourse.bass_utils import run_bass_kernel_spmd

    len1 = np.asarray(len1).astype(np.int64)
    len2 = np.asarray(len2).astype(np.int64)
    plan, nc, rm_col, cm_col = _get_plan_nc(len1, len2)
    in_maps, l1, l2 = _host_prep(reps1, reps2, len1, len2, plan)
    res = run_bass_kernel_spmd(nc, in_maps, list(range(NCORES)))
    LAST_RUN["results"] = res
    LAST_RUN["in_maps"] = in_maps
    LAST_RUN["nc"] = nc
    return _epilogue(res.results, l1, l2, w, b, plan, rm_col, cm_col)


# revision 41
# speedup vs baseline: 6.1735x; 1.1753x over previous
"""BertScore model kernel for Trainium2 (8 NeuronCores, SPMD, length-specialized).

Reference: cosine-normalized per-layer token reps, per-(layer,batch)
similarity matrix dots = h1 @ h2^T (L1 x L2, contraction D=1024), ragged
max over valid rows/cols + means -> s1,s2, F1 harmonic mean -> (B,NL)
features, BatchNorm over batch, linear head -> (B,).

Strategy: the 256 (layer,batch) units are independent until the host-side
BatchNorm. Units are grouped into 32 SPMD "slots" of 8 (one per core) with
similar (len1,len2); the device program is compiled for the actual lengths
(slot shape = max lens over its 8 units, rounded to 16 for the dual-fp8
LDWEIGHTS stride-alignment ISA rule), so only the valid ragged region is
transferred and computed. Host-side replicate-padding (row/col len-1 copied
into the padded tail) keeps every max exact with no masking. Inputs are fp8
e4m3 (end-to-end rel err ~8e-3 vs the 2e-2 gate; DMA is the roofline and
fp8 halves it); matmuls run in DoubleRow perf mode (two 128-deep K-tiles
per instruction, 0.5 cyc/row).

Per slot (stationary = the shorter of h1/h2, halving LDWEIGHTS cost):
4*n_st DoubleRow matmuls -> ACT copies PSUM to SBUF as f16 -> DVE free-axis
max over the moving index -> PE transposes (f16, software-pipelined one slot
behind the matmuls so the in-order PE queue never stalls on the ACT copy) ->
DVE free-axis max over the stationary index. Row/col max vectors accumulate
in SBUF f16 columns; two small DMAs out at the end. Slots execute
largest-first (minimal compute tail) and are DMA'd in multi-slot chunks
(first chunks small for fast pipeline fill) laid out so each partition reads
one contiguous run per chunk (~330 GB/s measured). Host epilogue: means
over valid prefixes, F1, BatchNorm over the full batch, linear head.

Measured on trn2 (8 cores, NTFF profile of the single NEFF execution):
55.7-68 us depending on device load (median ~58 us in quiet conditions),
vs 295 us for the staged baseline and 131 us for its single-shot profile.
"""
import os
import numpy as np

NL, B, L1, L2, D = 4, 64, 256, 256, 1024
NCORES = 8
NUNITS = NL * B           # 256 independent (layer, batch) units
NSLOTS = NUNITS // NCORES  # 32 slots, one unit per core each
KT = D // 128             # contraction tiles
BN_EPS = 1e-8
LOGIT_SCALE = 1.0

CHUNK_BYTES = int(os.environ.get("BSM_CHUNK", str(8 * 1024)))  # per partition

_CACHE = {}


def _plan(len1, len2):
    """Group units into slots; chunk slots for DMA; exec order big-first."""
    l1u = np.repeat(len1[None, :], NL, 0).ravel()   # unit u = l*B + b
    l2u = np.repeat(len2[None, :], NL, 0).ravel()
    nit = -(-l1u // 128)
    key = nit * 10**9 + l2u * 10**3 + l1u
    order_units = np.argsort(-key)                  # big first
    groups = order_units.reshape(NSLOTS, NCORES)
    # dual-fp8 LDWEIGHTS requires 16B-aligned k-tile strides
    shapes = np.stack([
        np.array([-16 * (-l1u[g].max() // 16) for g in groups]),
        np.array([-16 * (-l2u[g].max() // 16) for g in groups])], axis=1)
    order = np.argsort(-(shapes[:, 0] + shapes[:, 1]), kind="stable")
    # chunk consecutive exec-order slots: per-partition run per slot is
    # 8*(L1s+L2s) bytes. First chunks are small so compute starts early
    # (pipeline fill), later ones large (fewer DMA issues).
    ramp = [4 * 1024, 6 * 1024, 8 * 1024]
    chunks = []
    cur, cur_bytes = [], 0
    for s in order:
        cap = ramp[len(chunks)] if len(chunks) < len(ramp) else CHUNK_BYTES
        run = 8 * int(shapes[s][0] + shapes[s][1])
        if cur and cur_bytes + run > cap:
            chunks.append(cur)
            cur, cur_bytes = [], 0
        cur.append(int(s))
        cur_bytes += run
    if cur:
        chunks.append(cur)
    return {"groups": groups, "shapes": shapes, "order": order,
            "chunks": chunks}


def _build(shapes, chunks, plan):
    import concourse.bacc as bacc
    import concourse.bass as bass
    import concourse.mybir as mybir
    import concourse.tile as tile

    f32 = mybir.dt.float32
    f16 = mybir.dt.float16
    fp8 = mybir.dt.float8e4
    SWI = mybir.MatmulPerfMode.DoubleRowSwInterleave
    DR = mybir.MatmulPerfMode.DoubleRow

    # DRAM layout: chunk-major; within a chunk partition p holds one
    # contiguous run = concat over the chunk's slots of (8 d-rows of h1,
    # 8 d-rows of h2), each slot contributing 8*(L1s+L2s) bytes.
    chunk_off, chunk_run = [], []
    slot_in_chunk = {}
    off = 0
    for ci, ch in enumerate(chunks):
        run = 0
        for s in ch:
            slot_in_chunk[s] = (ci, run)
            run += 8 * int(shapes[s][0] + shapes[s][1])
        chunk_off.append(off)
        chunk_run.append(run)
        off += 128 * run
    TOT = off

    # RM: one f16 column per (slot, it); CM: one per (slot, jt).
    rm_col, cm_col = {}, {}
    nc1 = nc2 = 0
    for ch in chunks:
        for s in ch:
            L1s, L2s = int(shapes[s][0]), int(shapes[s][1])
            for it in range(-(-L1s // 128)):
                rm_col[(s, it)] = nc1
                nc1 += 1
            for jt in range(-(-L2s // 128)):
                cm_col[(s, jt)] = nc2
                nc2 += 1

    nc = bacc.Bacc("TRN2", target_bir_lowering=False, debug=False,
                   num_devices=NCORES)
    xin = nc.dram_tensor("xin", [TOT], fp8, kind="ExternalInput")
    rmd = nc.dram_tensor("rm", [128, nc1], f16, kind="ExternalOutput")
    cmd = nc.dram_tensor("cm", [128, nc2], f16, kind="ExternalOutput")
    xap = xin.ap()

    # quarters: contiguous chunk groups; each gets its own accumulator
    # tiles, DMA'd out as soon as its reduces are done (overlapped drain)
    NQ = 4
    nch = len(chunks)
    chunk_q = [min(NQ - 1, ci * NQ // nch) for ci in range(nch)]
    slot_q = {}
    for ci, ch in enumerate(chunks):
        for s in ch:
            slot_q[s] = chunk_q[ci]
    q_rm, q_cm = {}, {}   # quarter -> (col_lo, col_hi)
    for ci, ch in enumerate(chunks):
        q = chunk_q[ci]
        for s in ch:
            for t in range(-(-int(shapes[s][0]) // 128)):
                c = rm_col[(s, t)]
                lo, hi = q_rm.get(q, (c, c + 1))
                q_rm[q] = (min(lo, c), max(hi, c + 1))
            for t in range(-(-int(shapes[s][1]) // 128)):
                c = cm_col[(s, t)]
                lo, hi = q_cm.get(q, (c, c + 1))
                q_cm[q] = (min(lo, c), max(hi, c + 1))
    last_slot_of_q = {}
    for ci, ch in enumerate(chunks):
        last_slot_of_q[chunk_q[ci]] = ch[-1]
    last_slot_q = {s: q for q, s in last_slot_of_q.items()}

    with tile.TileContext(nc) as tc:
        from concourse.masks import make_identity
        with tc.tile_pool(name="consts", bufs=1) as consts, \
             tc.tile_pool(name="io", bufs=4) as io, \
             tc.tile_pool(name="dsbp", bufs=6) as dsbp, \
             tc.tile_pool(name="accp", bufs=1) as accp, \
             tc.tile_pool(name="ps", bufs=4, space="PSUM") as ps, \
             tc.tile_pool(name="psT", bufs=4, space="PSUM") as psT:

            ident = consts.tile([128, 128], f16)
            make_identity(nc, ident)
            RM = accp.tile([128, nc1], f16)
            CM = accp.tile([128, nc2], f16)

            vmax = mybir.AluOpType.max
            X = mybir.AxisListType.X
            IDENT = mybir.ActivationFunctionType.Identity

            MAXRUN = max(chunk_run)

            def emit_transposes(st):
                """Transpose phase of a slot: dT blocks + moving-side max."""
                s, Lst, Lmv, dsb, ilens, tr_acc, lc = st
                n_mv = -(-Lmv // 128)
                dT = psT.tile([128, 2, L1], f16, tag="dT")
                for tt in range(n_mv):
                    j0 = tt * 128
                    jlen = min(128, Lmv - j0)
                    for t, ilen in enumerate(ilens):
                        nc.tensor.transpose(
                            out=dT[:jlen, tt, t * 128:t * 128 + ilen],
                            in_=dsb[:ilen, t, j0:j0 + jlen],
                            identity=ident[:ilen, :ilen])
                if n_mv == 2:
                    # one merged reduce; tt=1 rows past jlen are garbage and
                    # discarded host-side (reduction is per-partition)
                    nc.vector.tensor_reduce(
                        out=tr_acc[:, lc:lc + 2], in_=dT[:, :, :Lst],
                        axis=X, op=vmax)
                else:
                    jlen = min(128, Lmv)
                    nc.vector.tensor_reduce(
                        out=tr_acc[:jlen, lc:lc + 1], in_=dT[:jlen, 0, :Lst],
                        axis=X, op=vmax)

            pending = None   # software pipeline: transposes lag one slot
            for ci, ch in enumerate(chunks):
                blk = io.tile([128, MAXRUN], fp8, tag="io")
                run = chunk_run[ci]
                nc.sync.dma_start(
                    out=blk[:, :run],
                    in_=bass.AP(tensor=xap.tensor, offset=chunk_off[ci],
                                ap=[[run, 128], [1, run]]))
                for s in ch:
                    L1s, L2s = int(shapes[s][0]), int(shapes[s][1])
                    soff = slot_in_chunk[s][1]
                    # stationary = shorter side (LDWEIGHTS cost ~ 8*Lst);
                    # free-axis reduce covers the stationary index,
                    # transposes cover the moving index.
                    if L1s <= L2s:
                        Lst, Lmv = L1s, L2s
                        fr_acc, frc = RM, rm_col[(s, 0)]
                        tr_acc, trc = CM, cm_col[(s, 0)]
                    else:
                        Lst, Lmv = L2s, L1s
                        fr_acc, frc = CM, cm_col[(s, 0)]
                        tr_acc, trc = RM, rm_col[(s, 0)]
                    stv = blk[:, soff:soff + 8 * Lst].rearrange(
                        "p (q i) -> p q i", q=8)
                    mvv = blk[:, soff + 8 * Lst:soff + 8 * (Lst + Lmv)
                              ].rearrange("p (q j) -> p q j", q=8)
                    n_st = -(-Lst // 128)

                    dsb = dsbp.tile([128, 2, L2], f16, tag="dsb")
                    ilens = []
                    for t in range(n_st):
                        i0 = t * 128
                        ilen = min(128, Lst - i0)
                        dps = ps.tile([128, L2], f32, tag="dots")
                        for k in range(0, KT, 2):
                            nc.tensor.matmul(
                                out=dps[:ilen, :Lmv],
                                lhsT=stv[:, k:k + 2, i0:i0 + ilen],
                                rhs=mvv[:, k:k + 2, :],
                                start=(k == 0), stop=(k == KT - 2),
                                perf_mode=DR)
                        nc.scalar.activation(
                            out=dsb[:ilen, t, :Lmv], in_=dps[:ilen, :Lmv],
                            func=IDENT)
                        ilens.append(ilen)
                    if n_st == 2:
                        # merged reduce; tile-1 rows past its ilen are
                        # garbage and discarded host-side
                        nc.vector.tensor_reduce(
                            out=fr_acc[:, frc:frc + 2],
                            in_=dsb[:, :, :Lmv], axis=X, op=vmax)
                    else:
                        nc.vector.tensor_reduce(
                            out=fr_acc[:ilens[0], frc:frc + 1],
                            in_=dsb[:ilens[0], 0, :Lmv], axis=X, op=vmax)

                    if pending is not None:
                        emit_transposes(pending)
                    pending = (s, Lst, Lmv, dsb, ilens, tr_acc, trc)
            if pending is not None:
                emit_transposes(pending)
            nc.sync.dma_start(out=rmd.ap(), in_=RM)
            nc.sync.dma_start(out=cmd.ap(), in_=CM)

    nc.finalize()
    return nc, rm_col, cm_col


def _get_plan_nc(len1, len2):
    key = (tuple(len1.tolist()), tuple(len2.tolist()), CHUNK_BYTES)
    if key not in _CACHE:
        plan = _plan(len1, len2)
        nc, rm_col, cm_col = _build(plan["shapes"], plan["chunks"], plan)
        _CACHE[key] = (plan, nc, rm_col, cm_col)
    return _CACHE[key]


def _host_prep(reps1, reps2, len1, len2, plan):
    """Normalize, fp8-cast, replicate-pad, pack per-core chunk buffers."""
    import ml_dtypes
    np_in = ml_dtypes.float8_e4m3

    def prep(r, lens, L):
        r = np.asarray(r, dtype=np.float32)
        n = np.sqrt(np.einsum('lbid,lbid->lbi', r, r))
        h = r / n[..., None]
        idx = np.minimum(np.arange(L)[None, :], (lens - 1)[:, None])  # (B, L)
        h = np.take_along_axis(h, idx[None, :, :, None], axis=2)
        return np.ascontiguousarray(h.transpose(0, 1, 3, 2)).astype(np_in)

    h1t = prep(reps1, len1, L1)   # (NL, B, D, L)
    h2t = prep(reps2, len2, L2)

    def swi_pack(h, Lst):
        """[1024, Lst] d-major -> [128, 8*Lst] (q-major per partition)."""
        return h.reshape(128, 8 * Lst)

    groups, shapes = plan["groups"], plan["shapes"]
    in_maps = []
    for c in range(NCORES):
        parts = []
        for ch in plan["chunks"]:
            rows = []   # per-partition segments, list of (128, seg) arrays
            for s in ch:
                u = groups[s][c]
                l, b = int(u) // B, int(u) % B
                L1s, L2s = int(shapes[s][0]), int(shapes[s][1])
                b1 = h1t[l, b, :, :L1s]
                b2 = h2t[l, b, :, :L2s]
                if L1s <= L2s:
                    st, mv, Lst = b1, b2, L1s
                else:
                    st, mv, Lst = b2, b1, L2s
                rows.append(swi_pack(st, Lst))
                rows.append(mv.reshape(128, 8 * mv.shape[1]))
            parts.append(np.concatenate(rows, axis=1).ravel())
        in_maps.append({"xin": np.concatenate(parts)})
    return in_maps, len1, len2


def _epilogue(results, len1, len2, w, b, plan, rm_col, cm_col):
    groups, shapes = plan["groups"], plan["shapes"]
    maxv_rows = np.zeros((NL, B, L1), dtype=np.float64)
    maxv_cols = np.zeros((NL, B, L2), dtype=np.float64)
    for c, res in enumerate(results):
        rm = np.asarray(res["rm"], dtype=np.float64)  # (128, nc1)
        cm = np.asarray(res["cm"], dtype=np.float64)
        for s in range(NSLOTS):
            u = groups[s][c]
            l, bb = int(u) // B, int(u) % B
            L1s, L2s = int(shapes[s][0]), int(shapes[s][1])
            for it in range(-(-L1s // 128)):
                ilen = min(128, L1s - it * 128)
                maxv_rows[l, bb, it * 128:it * 128 + ilen] = \
                    rm[:ilen, rm_col[(s, it)]]
            for jt in range(-(-L2s // 128)):
                jlen = min(128, L2s - jt * 128)
                maxv_cols[l, bb, jt * 128:jt * 128 + jlen] = \
                    cm[:jlen, cm_col[(s, jt)]]

    ar1 = np.arange(L1)[None, :]
    ar2 = np.arange(L2)[None, :]
    mask1 = (ar1 < len1[:, None])
    mask2 = (ar2 < len2[:, None])
    n1 = len1.astype(np.float64)
    n2 = len2.astype(np.float64)
    s2 = np.where(mask1[None], maxv_rows, 0.0).sum(axis=2) / n1[None]
    s1 = np.where(mask2[None], maxv_cols, 0.0).sum(axis=2) / n2[None]
    feat = (2.0 * s1 * s2 / (s1 + s2)).T
    mean = feat.mean(axis=0, keepdims=True)
    var = ((feat - mean) ** 2).mean(axis=0, keepdims=True)
    feat = (feat - mean) / np.sqrt(var + BN_EPS)
    w = np.asarray(w, dtype=np.float64)
    bb = np.asarray(b, dtype=np.float64)
    out = LOGIT_SCALE * (feat @ w.T + bb)[:, 0]
    return out.astype(np.float32)


LAST_RUN = {}


def kernel(reps1, reps2, len1, len2, w, b):
    from concourse.bass_utils import run_bass_kernel_spmd

    len1 = np.asarray(len1).astype(np.int64)
    len2 = np.asarray(len2).astype(np.int64)
    plan, nc, rm_col, cm_col = _get_plan_nc(len1, len2)
    in_maps, l1, l2 = _host_prep(reps1, reps2, len1, len2, plan)
    res = run_bass_kernel_spmd(nc, in_maps, list(range(NCORES)))
    LAST_RUN["results"] = res
    LAST_RUN["in_maps"] = in_maps
    LAST_RUN["nc"] = nc
    return _epilogue(res.results, l1, l2, w, b, plan, rm_col, cm_col)
